# revision 19
# baseline (speedup 1.0000x reference)
"""AttentionMIL pooling kernel for 8 Trainium2 NeuronCores.

Math (per slide b): h = tanh(X @ W1^T); s = h @ w2; a = softmax(s);
out = a^T @ X, with X [N=8192, D=1024], W1 [H=256, D], w2 [H].

Strategy (single-ship + engine-split weighted sum):
  - Data-parallel over slides: 16 slides / 8 cores = 2 per core.
  - X is shipped ONCE per core in the transposed layout xt (d on
    partitions), bf16: 32 MiB/core of HBM instead of 64.
  - h is computed FLIPPED: stationary = W1 chunks (reused), moving = xt,
    giving h^T [h on partitions, n free] in PSUM; ONE tanh per tile.
  - Scores: s[1, n] = sum_hh w2_hh^T @ tanh_hh — two tiny PE matmuls
    producing the score ROW directly; exp on ACT with accum_out giving
    the softmax denominator per tile for free (summed on host).
  - Softmax needs no max pass: |s| <= ||w2||_1 so exp cannot overflow.
  - Weighted sum out = sum_n e_n x_n is split across otherwise-idle
    engine pipes, d-chunk granular:
      * NXS chunks on the PE LoadStationary pipe: natural-layout blocks
        (built on-device by DMA XBAR transpose, SBUF->SBUF) loaded as
        stationary, e-column moving (1 streaming col per block).
        e-columns come from K=1 PE transpose matmuls of the e-row.
      * NDVE chunks on DVE via fused scalar_tensor_tensor
        (xt * e_bcast -> accum_out), e_bcast built by a K=1 PE outer
        product (ones^T @ e_row) read straight from PSUM.
    Per-tile partial sums land in SBUF/PSUM and are reduced on host.
"""

import sys

sys.path.insert(0, "/opt/trn_rl_repo")

import numpy as np
import ml_dtypes

import concourse.bacc as bacc
import concourse.tile as tile
from concourse import mybir
from concourse.bass_utils import run_bass_kernel_spmd

BF16 = ml_dtypes.bfloat16
B, N, D, H = 16, 8192, 1024, 256
NCORES = 8
SPC = B // NCORES          # slides per core
NT = 512                   # rows of N per tile
TILES = N // NT
KCH = D // 128             # d-chunks of 128
NXS = 1                    # d-chunks on the PE X-as-stationary path (steady)
NXS_TAIL = 5               # tail tiles go XS-heavy so the drain isn't DVE-paced
NTAIL = 4                  # number of trailing global tiles using NXS_TAIL
NDVE = KCH - NXS


def nxs_of(i):
    return NXS_TAIL if i >= SPC * TILES - NTAIL else NXS

_NC_CACHE = {}


def _build_nc():
    bf = mybir.dt.bfloat16
    f32 = mybir.dt.float32
    AF = mybir.ActivationFunctionType
    MUL = mybir.AluOpType.mult

    nc = bacc.Bacc("TRN2", num_devices=NCORES)
    # xt[s, t, p, k*NT + j] = X[s, t*NT + j, k*128 + p]
    xt = nc.declare_dram_parameter("xt", [SPC, TILES, 128, KCH * NT], bf, isOutput=False)
    # xn[s, t, p, (k*4+blk)*128 + i] = X[s, t*NT + blk*128 + p, k*128 + i]
    xn = nc.declare_dram_parameter("xn", [SPC, TILES, 128, NXS_TAIL * 4 * 128], bf, isOutput=False)
    # w1s[p, (k*2+hh)*128 + j] = W1[hh*128 + j, k*128 + p]
    w1s = nc.declare_dram_parameter("w1s", [128, KCH * 2 * 128], bf, isOutput=False)
    # w2c[p, hh] = W2[0, hh*128 + p]
    w2c = nc.declare_dram_parameter("w2c", [128, 2], bf, isOutput=False)
    ones = nc.declare_dram_parameter("ones", [1, 128], bf, isOutput=False)
    xsacc = nc.declare_dram_parameter("xsacc", [SPC, 128, TILES * NXS_TAIL], f32, isOutput=True)
    wacc = nc.declare_dram_parameter("wacc", [SPC, 128, TILES * NDVE], f32, isOutput=True)
    lrow = nc.declare_dram_parameter("lrow", [SPC, 1, TILES], f32, isOutput=True)

    with tile.TileContext(nc) as tc:
        with tc.tile_pool(name="const", bufs=1) as constp, \
             tc.tile_pool(name="xt", bufs=12) as xtp, \
             tc.tile_pool(name="xn", bufs=10) as xnp, \
             tc.tile_pool(name="th", bufs=4) as thp, \
             tc.tile_pool(name="erow", bufs=4) as erp, \
             tc.tile_pool(name="ecol", bufs=3) as ecp, \
             tc.tile_pool(name="scr", bufs=1) as scrp, \
             tc.tile_pool(name="acc", bufs=1) as accp, \
             tc.tile_pool(name="out", bufs=2) as outp, \
             tc.tile_pool(name="hps", bufs=2, space="PSUM") as hpsp, \
             tc.tile_pool(name="sps", bufs=1, space="PSUM") as spsp, \
             tc.tile_pool(name="ebcps", bufs=2, space="PSUM") as ebcpsp, \
             tc.tile_pool(name="ecxs", bufs=1, space="PSUM") as ecxsp:

            warm_sb = constp.tile([128, 256], bf)
            nc.gpsimd.memset(warm_sb[:], 0.0)
            w1s_sb = constp.tile([128, KCH * 2 * 128], bf)
            nc.gpsimd.dma_start(w1s_sb[:], w1s[:, :])
            w2c_sb = constp.tile([128, 2], bf)
            nc.gpsimd.dma_start(w2c_sb[:], w2c[:, :])
            ones_sb = constp.tile([1, 128], bf)
            nc.gpsimd.dma_start(ones_sb[:], ones[:, :])

            warm_ps = hpsp.tile([128, 2 * NT], f32, tag="h_ps")
            for _ in range(40):
                nc.tensor.matmul(
                    warm_ps[:, 0:H], warm_sb[:, 0:128], warm_sb[:, 0:H],
                    start=True, stop=True, skip_group_check=True,
                )

            l_sbs, wacc_sbs, xsout_sbs = [], [], []
            for s in range(SPC):
                l_sbs.append(accp.tile([1, TILES], f32, tag=f"l_sb{s}", name=f"l_sb{s}"))
                wacc_sbs.append(accp.tile([128, TILES * NDVE], f32, tag=f"wacc{s}", name=f"wacc{s}"))
                xsout_sbs.append(accp.tile([128, TILES * NXS_TAIL], f32, tag=f"xsout{s}", name=f"xsout{s}"))

            state = {}

            def stage_A0(i):
                s, t = divmod(i, TILES)
                xt_sb = xtp.tile([128, KCH * NT], bf)
                nc.sync.dma_start(xt_sb[:], xt[s, t])
                nxs = nxs_of(i)
                xn_sb = xnp.tile([128, NXS_TAIL * 4, 128], bf)
                nc.sync.dma_start(
                    xn_sb[:, 0:nxs * 4, :].rearrange("p a b -> p (a b)"),
                    xn[s, t, :, 0:nxs * 4 * 128])
                state[i] = (xt_sb, xn_sb)

            def stage_A1(i):
                (xt_sb, xn_sb) = state[i]
                h_ps = hpsp.tile([128, 2 * NT], f32, tag="h_ps")
                for hh in range(2):
                    for k in range(KCH):
                        nc.tensor.matmul(
                            h_ps[:, hh * NT:(hh + 1) * NT],
                            w1s_sb[:, (k * 2 + hh) * 128:(k * 2 + hh + 1) * 128],
                            xt_sb[:, k * NT:(k + 1) * NT],
                            start=(k == 0), stop=(k == KCH - 1),
                        )
                th_sb = thp.tile([128, 2 * NT], bf)
                nc.scalar.activation(th_sb[:], h_ps[:], AF.Tanh)
                state[i] = state[i] + (th_sb,)

            def stage_B(i):
                s, t = divmod(i, TILES)
                th_sb = state[i][2]
                s_ps = spsp.tile([1, NT], f32)
                for hh in range(2):
                    nc.tensor.matmul(
                        s_ps[:], w2c_sb[:, hh:hh + 1],
                        th_sb[:, hh * NT:(hh + 1) * NT],
                        start=(hh == 0), stop=(hh == 1),
                    )
                e_row = erp.tile([1, NT], bf)
                nc.scalar.activation(
                    e_row[:], s_ps[:], AF.Exp,
                    accum_out=l_sbs[s][0:1, t:t + 1])
                state[i] = state[i] + (e_row,)

            def stage_C(i):
                s, t = divmod(i, TILES)
                xt_sb, xn_sb, th_sb, e_row = state.pop(i)
                ebc_ps = ebcpsp.tile([128, NT], f32)
                nc.tensor.matmul(
                    ebc_ps[:], ones_sb[:], e_row[:],
                    start=True, stop=True, skip_group_check=True)
                nxs = nxs_of(i)
                ndve = KCH - nxs
                ecxs_ps = ecxsp.tile([128, 4 + NXS_TAIL], f32)
                ec_ps = ecxs_ps
                for blk in range(4):
                    nc.tensor.matmul(
                        ec_ps[:, blk:blk + 1],
                        e_row[0:1, blk * 128:(blk + 1) * 128],
                        ones_sb[0:1, 0:1],
                        start=True, stop=True, skip_group_check=True)
                ecol_sb = ecp.tile([128, 4], bf)
                nc.scalar.activation(ecol_sb[:], ec_ps[:, 0:4], AF.Copy)
                for k in range(nxs):
                    for blk in range(4):
                        nc.tensor.matmul(
                            ecxs_ps[:, 4 + k:4 + k + 1],
                            xn_sb[:, k * 4 + blk, :],
                            ecol_sb[:, blk:blk + 1],
                            start=(blk == 0),
                            stop=(blk == 3),
                            skip_group_check=True)
                nc.scalar.activation(
                    xsout_sbs[s][:, t * NXS_TAIL:t * NXS_TAIL + nxs],
                    ecxs_ps[:, 4:4 + nxs], AF.Copy)
                scr = scrp.tile([128, NT], bf, tag="scr")
                for ii in range(ndve):
                    k = nxs + ii
                    nc.vector.scalar_tensor_tensor(
                        out=scr[:], in0=xt_sb[:, k * NT:(k + 1) * NT],
                        scalar=1.0, in1=ebc_ps[:],
                        op0=MUL, op1=MUL,
                        accum_out=wacc_sbs[s][:, t * NDVE + ii:t * NDVE + ii + 1])
                if t == TILES - 1:
                    nc.gpsimd.dma_start(xsacc[s], xsout_sbs[s][:])
                    nc.gpsimd.dma_start(wacc[s], wacc_sbs[s][:])
                    nc.gpsimd.dma_start(lrow[s], l_sbs[s][:])

            NTOT = SPC * TILES
            for tau in range(NTOT + 7):
                if 0 <= tau - 6 < NTOT:
                    stage_B(tau - 6)
                if 0 <= tau - 7 < NTOT:
                    stage_C(tau - 7)
                if tau < NTOT:
                    stage_A0(tau)
                if 0 <= tau - 4 < NTOT:
                    stage_A1(tau - 4)

    nc.compile()
    return nc


def _get_nc():
    if "nc" not in _NC_CACHE:
        _NC_CACHE["nc"] = _build_nc()
    return _NC_CACHE["nc"]


def _prep_inputs(tiles_embeddings, W1, W2):
    X_bf = tiles_embeddings.astype(BF16)
    # xt[b, t, p, k, j] = X[b, t*NT + j, k*128 + p]
    xt_sw = np.ascontiguousarray(
        X_bf.reshape(B, TILES, NT, KCH, 128).transpose(0, 1, 4, 3, 2)
    ).reshape(B, TILES, 128, KCH * NT)
    # xn2[b, t, p, k, blk, i] = X[b, t*NT + blk*128 + p, k*128 + i]
    xn_sw = np.ascontiguousarray(
        X_bf.reshape(B, TILES, 4, 128, KCH, 128)[:, :, :, :, :NXS_TAIL]
        .transpose(0, 1, 3, 4, 2, 5)
    ).reshape(B, TILES, 128, NXS_TAIL * 4 * 128)
    # w1s[p, k, hh, j] = W1[hh*128 + j, k*128 + p]
    w1s = np.ascontiguousarray(
        W1.astype(BF16).reshape(2, 128, KCH, 128).transpose(3, 2, 0, 1)
    ).reshape(128, KCH * 2 * 128)
    w2c = np.ascontiguousarray(W2.astype(BF16).reshape(2, 128).T)
    ones = np.ones((1, 128), BF16)
    return [
        {
            "xt": xt_sw[c * SPC:(c + 1) * SPC],
            "xn": xn_sw[c * SPC:(c + 1) * SPC],
            "w1s": w1s,
            "w2c": w2c,
            "ones": ones,
        }
        for c in range(NCORES)
    ]


def _run(tiles_embeddings, W1, W2, **spmd_kwargs):
    nc = _get_nc()
    in_maps = _prep_inputs(tiles_embeddings, W1, W2)
    res = run_bass_kernel_spmd(nc, in_maps, core_ids=list(range(NCORES)), **spmd_kwargs)
    out = np.empty((B, D), np.float32)
    for c in range(NCORES):
        r = res.results[c]
        for s in range(SPC):
            b = c * SPC + s
            acc = np.zeros((128, KCH), np.float32)
            xsr = r["xsacc"][s].reshape(128, TILES, NXS_TAIL)
            war = r["wacc"][s].reshape(128, TILES, NDVE)
            for t in range(TILES):
                nxs = nxs_of(s * TILES + t)
                acc[:, :nxs] += xsr[:, t, :nxs]
                acc[:, nxs:] += war[:, t, :KCH - nxs]
            l = r["lrow"][s].sum()
            out[b] = (acc.T.reshape(D) / l)
    return out, res


def kernel(tiles_embeddings, W1, W2):
    out, _ = _run(
        np.asarray(tiles_embeddings), np.asarray(W1), np.asarray(W2)
    )
    return out


# revision 20
# speedup vs baseline: 1.0709x; 1.0709x over previous
"""AttentionMIL pooling kernel for 8 Trainium2 NeuronCores.

Math (per slide b): h = tanh(X @ W1^T); s = h @ w2; a = softmax(s);
out = a^T @ X, with X [N=8192, D=1024], W1 [H=256, D], w2 [H].

Strategy (single-ship + engine-split weighted sum):
  - Data-parallel over slides: 16 slides / 8 cores = 2 per core.
  - X is shipped ONCE per core in the transposed layout xt (d on
    partitions), bf16: 32 MiB/core of HBM instead of 64.
  - h is computed FLIPPED: stationary = W1 chunks (reused), moving = xt,
    giving h^T [h on partitions, n free] in PSUM; ONE tanh per tile.
  - Scores: s[1, n] = sum_hh w2_hh^T @ tanh_hh — two tiny PE matmuls
    producing the score ROW directly; exp on ACT with accum_out giving
    the softmax denominator per tile for free (summed on host).
  - Softmax needs no max pass: |s| <= ||w2||_1 so exp cannot overflow.
  - Weighted sum out = sum_n e_n x_n is split across otherwise-idle
    engine pipes, d-chunk granular:
      * NXS chunks on the PE LoadStationary pipe: natural-layout blocks
        (built on-device by DMA XBAR transpose, SBUF->SBUF) loaded as
        stationary, e-column moving (1 streaming col per block).
        e-columns come from K=1 PE transpose matmuls of the e-row.
      * NDVE chunks on DVE via fused scalar_tensor_tensor
        (xt * e_bcast -> accum_out), e_bcast built by a K=1 PE outer
        product (ones^T @ e_row) read straight from PSUM.
    Per-tile partial sums land in SBUF/PSUM and are reduced on host.
"""

import sys

sys.path.insert(0, "/opt/trn_rl_repo")

import numpy as np
import ml_dtypes

import concourse.bacc as bacc
import concourse.tile as tile
from concourse import mybir
from concourse.bass_utils import run_bass_kernel_spmd

BF16 = ml_dtypes.bfloat16
B, N, D, H = 16, 8192, 1024, 256
NCORES = 8
SPC = B // NCORES          # slides per core
NT = 512                   # rows of N per tile
TILES = N // NT
KCH = D // 128             # d-chunks of 128
NXS = 2                    # d-chunks on the PE X-as-stationary path (steady)
NXS_TAIL = 5               # tail tiles go XS-heavy so the drain isn't DVE-paced
NTAIL = 4                  # number of trailing global tiles using NXS_TAIL
NDVE = KCH - NXS


def nxs_of(i):
    return NXS_TAIL if i >= SPC * TILES - NTAIL else NXS

_NC_CACHE = {}


def _build_nc():
    bf = mybir.dt.bfloat16
    f32 = mybir.dt.float32
    AF = mybir.ActivationFunctionType
    MUL = mybir.AluOpType.mult

    nc = bacc.Bacc("TRN2", num_devices=NCORES)
    # xt[s, t, p, k*NT + j] = X[s, t*NT + j, k*128 + p]
    xt = nc.declare_dram_parameter("xt", [SPC, TILES, 128, KCH * NT], bf, isOutput=False)
    # xn[s, t, p, (k*4+blk)*128 + i] = X[s, t*NT + blk*128 + p, k*128 + i]
    xn = nc.declare_dram_parameter("xn", [SPC, TILES, 128, NXS_TAIL * 4 * 128], bf, isOutput=False)
    # w1s[p, (k*2+hh)*128 + j] = W1[hh*128 + j, k*128 + p]
    w1s = nc.declare_dram_parameter("w1s", [128, KCH * 2 * 128], bf, isOutput=False)
    # w2c[p, hh] = W2[0, hh*128 + p]
    w2c = nc.declare_dram_parameter("w2c", [128, 2], bf, isOutput=False)
    ones = nc.declare_dram_parameter("ones", [1, 128], bf, isOutput=False)
    xsacc = nc.declare_dram_parameter("xsacc", [SPC, 128, TILES * NXS_TAIL], f32, isOutput=True)
    wacc = nc.declare_dram_parameter("wacc", [SPC, 128, TILES * NDVE], f32, isOutput=True)
    lrow = nc.declare_dram_parameter("lrow", [SPC, 1, TILES], f32, isOutput=True)

    with tile.TileContext(nc) as tc:
        with tc.tile_pool(name="const", bufs=1) as constp, \
             tc.tile_pool(name="xt", bufs=12) as xtp, \
             tc.tile_pool(name="xn", bufs=10) as xnp, \
             tc.tile_pool(name="th", bufs=4) as thp, \
             tc.tile_pool(name="erow", bufs=4) as erp, \
             tc.tile_pool(name="ecol", bufs=3) as ecp, \
             tc.tile_pool(name="scr", bufs=1) as scrp, \
             tc.tile_pool(name="acc", bufs=1) as accp, \
             tc.tile_pool(name="out", bufs=2) as outp, \
             tc.tile_pool(name="hps", bufs=2, space="PSUM") as hpsp, \
             tc.tile_pool(name="sps", bufs=1, space="PSUM") as spsp, \
             tc.tile_pool(name="ebcps", bufs=2, space="PSUM") as ebcpsp, \
             tc.tile_pool(name="ecxs", bufs=1, space="PSUM") as ecxsp:

            warm_sb = constp.tile([128, 256], bf)
            nc.gpsimd.memset(warm_sb[:], 0.0)
            w1s_sb = constp.tile([128, KCH * 2 * 128], bf)
            nc.gpsimd.dma_start(w1s_sb[:], w1s[:, :])
            w2c_sb = constp.tile([128, 2], bf)
            nc.gpsimd.dma_start(w2c_sb[:], w2c[:, :])
            ones_sb = constp.tile([1, 128], bf)
            nc.gpsimd.dma_start(ones_sb[:], ones[:, :])

            warm_ps = hpsp.tile([128, 2 * NT], f32, tag="h_ps")
            for _ in range(40):
                nc.tensor.matmul(
                    warm_ps[:, 0:H], warm_sb[:, 0:128], warm_sb[:, 0:H],
                    start=True, stop=True, skip_group_check=True,
                )

            l_sbs, wacc_sbs, xsout_sbs = [], [], []
            for s in range(SPC):
                l_sbs.append(accp.tile([1, TILES], f32, tag=f"l_sb{s}", name=f"l_sb{s}"))
                wacc_sbs.append(accp.tile([128, TILES * NDVE], f32, tag=f"wacc{s}", name=f"wacc{s}"))
                xsout_sbs.append(accp.tile([128, TILES * NXS_TAIL], f32, tag=f"xsout{s}", name=f"xsout{s}"))

            state = {}

            def stage_A0(i):
                s, t = divmod(i, TILES)
                xt_sb = xtp.tile([128, KCH * NT], bf)
                nc.sync.dma_start(xt_sb[:], xt[s, t])
                nxs = nxs_of(i)
                xn_sb = xnp.tile([128, NXS_TAIL * 4, 128], bf)
                nc.sync.dma_start(
                    xn_sb[:, 0:nxs * 4, :].rearrange("p a b -> p (a b)"),
                    xn[s, t, :, 0:nxs * 4 * 128])
                state[i] = (xt_sb, xn_sb)

            def stage_A1(i):
                (xt_sb, xn_sb) = state[i]
                h_ps = hpsp.tile([128, 2 * NT], f32, tag="h_ps")
                for hh in range(2):
                    for k in range(KCH):
                        nc.tensor.matmul(
                            h_ps[:, hh * NT:(hh + 1) * NT],
                            w1s_sb[:, (k * 2 + hh) * 128:(k * 2 + hh + 1) * 128],
                            xt_sb[:, k * NT:(k + 1) * NT],
                            start=(k == 0), stop=(k == KCH - 1),
                        )
                th_sb = thp.tile([128, 2 * NT], bf)
                nc.scalar.activation(th_sb[:], h_ps[:], AF.Tanh)
                state[i] = state[i] + (th_sb,)

            def stage_B(i):
                s, t = divmod(i, TILES)
                th_sb = state[i][2]
                s_ps = spsp.tile([1, NT], f32)
                for hh in range(2):
                    nc.tensor.matmul(
                        s_ps[:], w2c_sb[:, hh:hh + 1],
                        th_sb[:, hh * NT:(hh + 1) * NT],
                        start=(hh == 0), stop=(hh == 1),
                    )
                e_row = erp.tile([1, NT], bf)
                nc.scalar.activation(
                    e_row[:], s_ps[:], AF.Exp,
                    accum_out=l_sbs[s][0:1, t:t + 1])
                state[i] = state[i] + (e_row,)

            def stage_C(i):
                s, t = divmod(i, TILES)
                xt_sb, xn_sb, th_sb, e_row = state.pop(i)
                ebc_ps = ebcpsp.tile([128, NT], f32)
                nc.tensor.matmul(
                    ebc_ps[:], ones_sb[:], e_row[:],
                    start=True, stop=True, skip_group_check=True)
                nxs = nxs_of(i)
                ndve = KCH - nxs
                ecxs_ps = ecxsp.tile([128, 4 + NXS_TAIL], f32)
                ec_ps = ecxs_ps
                for blk in range(4):
                    nc.tensor.matmul(
                        ec_ps[:, blk:blk + 1],
                        e_row[0:1, blk * 128:(blk + 1) * 128],
                        ones_sb[0:1, 0:1],
                        start=True, stop=True, skip_group_check=True)
                ecol_sb = ecp.tile([128, 4], bf)
                nc.scalar.activation(ecol_sb[:], ec_ps[:, 0:4], AF.Copy)
                for k in range(nxs):
                    for blk in range(4):
                        nc.tensor.matmul(
                            ecxs_ps[:, 4 + k:4 + k + 1],
                            xn_sb[:, k * 4 + blk, :],
                            ecol_sb[:, blk:blk + 1],
                            start=(blk == 0),
                            stop=(blk == 3),
                            skip_group_check=True)
                nc.scalar.activation(
                    xsout_sbs[s][:, t * NXS_TAIL:t * NXS_TAIL + nxs],
                    ecxs_ps[:, 4:4 + nxs], AF.Copy)
                scr = scrp.tile([128, NT], bf, tag="scr")
                for ii in range(ndve):
                    k = nxs + ii
                    nc.vector.scalar_tensor_tensor(
                        out=scr[:], in0=xt_sb[:, k * NT:(k + 1) * NT],
                        scalar=1.0, in1=ebc_ps[:],
                        op0=MUL, op1=MUL,
                        accum_out=wacc_sbs[s][:, t * NDVE + ii:t * NDVE + ii + 1])
                if t == TILES - 1:
                    nc.gpsimd.dma_start(xsacc[s], xsout_sbs[s][:])
                    nc.gpsimd.dma_start(wacc[s], wacc_sbs[s][:])
                    nc.gpsimd.dma_start(lrow[s], l_sbs[s][:])

            NTOT = SPC * TILES
            for tau in range(NTOT + 7):
                if 0 <= tau - 6 < NTOT:
                    stage_B(tau - 6)
                if 0 <= tau - 7 < NTOT:
                    stage_C(tau - 7)
                if tau < NTOT:
                    stage_A0(tau)
                if 0 <= tau - 4 < NTOT:
                    stage_A1(tau - 4)

    nc.compile()
    return nc


def _get_nc():
    if "nc" not in _NC_CACHE:
        _NC_CACHE["nc"] = _build_nc()
    return _NC_CACHE["nc"]


def _prep_inputs(tiles_embeddings, W1, W2):
    X_bf = tiles_embeddings.astype(BF16)
    # xt[b, t, p, k, j] = X[b, t*NT + j, k*128 + p]
    xt_sw = np.ascontiguousarray(
        X_bf.reshape(B, TILES, NT, KCH, 128).transpose(0, 1, 4, 3, 2)
    ).reshape(B, TILES, 128, KCH * NT)
    # xn2[b, t, p, k, blk, i] = X[b, t*NT + blk*128 + p, k*128 + i]
    xn_sw = np.ascontiguousarray(
        X_bf.reshape(B, TILES, 4, 128, KCH, 128)[:, :, :, :, :NXS_TAIL]
        .transpose(0, 1, 3, 4, 2, 5)
    ).reshape(B, TILES, 128, NXS_TAIL * 4 * 128)
    # w1s[p, k, hh, j] = W1[hh*128 + j, k*128 + p]
    w1s = np.ascontiguousarray(
        W1.astype(BF16).reshape(2, 128, KCH, 128).transpose(3, 2, 0, 1)
    ).reshape(128, KCH * 2 * 128)
    w2c = np.ascontiguousarray(W2.astype(BF16).reshape(2, 128).T)
    ones = np.ones((1, 128), BF16)
    return [
        {
            "xt": xt_sw[c * SPC:(c + 1) * SPC],
            "xn": xn_sw[c * SPC:(c + 1) * SPC],
            "w1s": w1s,
            "w2c": w2c,
            "ones": ones,
        }
        for c in range(NCORES)
    ]


def _run(tiles_embeddings, W1, W2, **spmd_kwargs):
    nc = _get_nc()
    in_maps = _prep_inputs(tiles_embeddings, W1, W2)
    res = run_bass_kernel_spmd(nc, in_maps, core_ids=list(range(NCORES)), **spmd_kwargs)
    out = np.empty((B, D), np.float32)
    for c in range(NCORES):
        r = res.results[c]
        for s in range(SPC):
            b = c * SPC + s
            acc = np.zeros((128, KCH), np.float32)
            xsr = r["xsacc"][s].reshape(128, TILES, NXS_TAIL)
            war = r["wacc"][s].reshape(128, TILES, NDVE)
            for t in range(TILES):
                nxs = nxs_of(s * TILES + t)
                acc[:, :nxs] += xsr[:, t, :nxs]
                acc[:, nxs:] += war[:, t, :KCH - nxs]
            l = r["lrow"][s].sum()
            out[b] = (acc.T.reshape(D) / l)
    return out, res


def kernel(tiles_embeddings, W1, W2):
    out, _ = _run(
        np.asarray(tiles_embeddings), np.asarray(W1), np.asarray(W2)
    )
    return out
